# revision 8
# baseline (speedup 1.0000x reference)
import sys
import itertools

for p in ("/opt/trn_rl_repo",):
    if p not in sys.path:
        sys.path.insert(0, p)

import numpy as np
import ml_dtypes

from concourse import bass, mybir, bacc, tile
from concourse.ap import AP
from concourse.bass_utils import run_bass_kernel_spmd


def _install_ntff_hook():
    try:
        from antenv import axon_hooks  # noqa: F401
        return
    except ImportError:
        pass
    import types
    try:
        import antenv
    except ImportError:
        return
    mod = types.ModuleType("antenv.axon_hooks")
    _h = {"hook": None}
    mod.set_axon_ntff_profile_hook = lambda h: _h.__setitem__("hook", h)
    mod.get_axon_ntff_profile_hook = lambda: _h["hook"]
    sys.modules["antenv.axon_hooks"] = mod
    antenv.axon_hooks = mod
    try:
        from trn_agent_boot.trn_boot import _ntff_profile_via_ctypes
        h = _ntff_profile_via_ctypes("/opt/axon/libaxon_pjrt.so")
        if h is not None:
            mod.set_axon_ntff_profile_hook(h)
    except Exception:
        pass


_install_ntff_hook()


F32 = mybir.dt.float32
BF16 = mybir.dt.bfloat16
FP8 = mybir.dt.float8e4
MUL = mybir.AluOpType.mult
ADD = mybir.AluOpType.add
AXX = mybir.AxisListType.X
EXP = mybir.ActivationFunctionType.Exp
COPY = mybir.ActivationFunctionType.Copy
DR = mybir.MatmulPerfMode.DoubleRow

B, C, H, W = 16, 256, 96, 96
S = H * W          # 9216
NCORE = 8
BPC = B // NCORE   # 2 batches per core
QKC = 66           # q(32) | k(32) | sigma(1) | pad(1)
PW = QKC + 256     # 322 proj width
HP = 128           # padded h for xbar transpose
WSCALE = 4.0       # wall rows prescale (fp8 range)
VSCALE = 16.0      # v_sb carries VSCALE*gamma*v


def _apv(t, off, dims):
    """Custom view on a tile/tensor AP: keep partition dim, custom free dims."""
    b = t[:] if not isinstance(t, AP) else t
    part = list(b.ap[0])
    return AP(b.tensor, b.offset + off, [part] + [list(d) for d in dims])


def build_graph(gamma):
    nc = bacc.Bacc(None, target_bir_lowering=False)
    vg = float(VSCALE * gamma / WSCALE)

    xa_e = nc.declare_dram_parameter("xa", [BPC, 128, 2, S], FP8, isOutput=False)
    wall_e = nc.declare_dram_parameter("wall", [128, 2, PW], FP8, isOutput=False)
    pstr_e = nc.declare_dram_parameter("pstr", [96, 96], BF16, isOutput=False)
    ipat_e = nc.declare_dram_parameter("ipat", [96, 864], BF16, isOutput=False)
    idtb_e = nc.declare_dram_parameter("idtb", [96, 96], BF16, isOutput=False)
    bvz_e = nc.declare_dram_parameter("bvz", [32, 96 * 256], FP8, isOutput=False)
    out_e = nc.declare_dram_parameter("out", [BPC, 2, 128, S], BF16, isOutput=True)

    with tile.TileContext(nc) as tc:
        with (
            tc.tile_pool(name="const", bufs=1) as cp,
            tc.tile_pool(name="main", bufs=1) as mp,
            tc.tile_pool(name="work", bufs=2) as wp,
            tc.tile_pool(name="pj", bufs=2, space="PSUM") as pj,
            tc.tile_pool(name="avp", bufs=2, space="PSUM") as avp,
        ):
            wall_sb = cp.tile([128, 2 * PW], FP8, tag="wall")
            nc.sync.dma_start(wall_sb[:], wall_e[:])
            pstr_sb = cp.tile([96, 96], BF16, tag="pstr")
            nc.sync.dma_start(pstr_sb[:], pstr_e[:])
            ipat_sb = cp.tile([96, 864], BF16, tag="ipat")
            nc.sync.dma_start(ipat_sb[:], ipat_e[:])
            idtb_sb = cp.tile([96, 96], BF16, tag="idtb")
            nc.sync.dma_start(idtb_sb[:], idtb_e[:])

            st = {0: {}, 1: {}}
            rhs_tiles = []
            for ri in range(2):
                rt = mp.tile([128, 2 * 864], FP8, tag=f"rhs{ri}")
                nc.vector.memset(rt[96:128, :], 0.0)
                rhs_tiles.append(rt)

            def stage_load(b):
                xa_sb = mp.tile([128, 2 * S], FP8, tag="xa", bufs=2,
                                name=f"xa{b}")
                nc.sync.dma_start(xa_sb[:], xa_e[b])
                st[b]["xa"] = xa_sb
                v_sb = mp.tile([128, 96 * 256], FP8, tag="v", bufs=2, name=f"v{b}")
                nc.sync.dma_start(v_sb[96:128, :], bvz_e[:])
                st[b]["v"] = v_sb
                qk_sb = mp.tile([96, QKC * HP], BF16, tag="qk", bufs=2,
                                name=f"qk{b}")
                st[b]["qk"] = qk_sb
                # pad rows h=96..127 of each channel (read by the xbar
                # transpose, discarded downstream)
                nc.gpsimd.memset(_apv(qk_sb, 96, [[HP, QKC], [1, HP - 96]]), 0.0)

            def stage_proj(b, gen=None):
                """proj: per 2 h-lines, one fp8 DoubleRow matmul each (cc pair
                fused as the two K-tiles); psum [96, 1024] (lines at col
                0/512); evict qk ch-major (ACT) + v (ACT/DVE split, fp8)."""
                xa_sb, qk_sb, v_sb = st[b]["xa"], st[b]["qk"], st[b]["v"]
                for g in range(48):
                    ps = pj.tile([96, 1024], F32, tag="pj", name=f"ps{b}_{g}")
                    for l2 in range(2):
                        h = 2 * g + l2
                        nc.tensor.matmul(
                            _apv(ps, 512 * l2, [[1, PW]]),
                            _apv(xa_sb, h * 96, [[S, 2], [1, 96]]),
                            _apv(wall_sb, 0, [[PW, 2], [1, PW]]),
                            start=True, stop=True,
                            perf_mode=DR,
                        )
                    # qk[w, c*HP + h] <- ps[w, l2*512 + c] / WSCALE
                    nc.scalar.activation(
                        _apv(qk_sb, 2 * g, [[HP, QKC], [1, 2]]),
                        _apv(ps, 0, [[1, QKC], [512, 2]]),
                        COPY, scale=1.0 / WSCALE,
                    )
                    if b == 0:
                        on_dve = g % 2 == 1
                    else:
                        on_dve = g % 8 == 7
                    if on_dve:
                        nc.vector.tensor_scalar_mul(
                            v_sb[0:96, g * 512:(g + 1) * 512],
                            _apv(ps, QKC, [[512, 2], [1, 256]]),
                            vg,
                        )
                    else:
                        nc.scalar.activation(
                            v_sb[0:96, g * 512:(g + 1) * 512],
                            _apv(ps, QKC, [[512, 2], [1, 256]]),
                            COPY, scale=vg,
                        )
                    if gen is not None and g % 2 == 1:
                        next(gen, None)
                if gen is not None:
                    for _ in gen:
                        pass

            def stage_qkT(b):
                """one XBAR dma-transpose: qk[w, c*HP+h] -> qkc[h, c*96+w]."""
                qk_sb = st[b]["qk"]
                qkc = mp.tile([128, QKC * 96], BF16, tag="qkc", bufs=2,
                              name=f"qkc{b}")
                st[b]["qkc"] = qkc
                nc.sync.dma_start_transpose(
                    _apv(qkc, 0, [[96, QKC], [1, 96]]),
                    qk_sb[:],
                )

            def scores_products_gen(b, nm):
                """nm='h': qk_sb [w, c*HP+h]; nm='v': qkc [h, c*96+w].
                products (DVE), reduces (DVE), sigma-add (GPS), exp (ACT),
                s3 (DVE), r3 (DVE), a (GPS), bias (DVE). Yields per pair."""
                if nm == "h":
                    src = st[b]["qk"][:]
                    cs = HP
                else:
                    src = st[b]["qkc"][0:96, :]
                    cs = 96
                qoff = lambda k: (k, [[3, 32], [cs, 32]])
                koff = lambda j: (32 * cs + j, [[3, 32], [cs, 32]])
                sig = (64 * cs, [[0, 3], [1, 3], [3, 32]])
                sraw = mp.tile([96, 288], F32, tag=f"sraw{nm}", name=f"sraw{nm}{b}")
                te = mp.tile([96, 288], F32, tag=f"te{nm}", name=f"te{nm}{b}")
                s3 = mp.tile([96, 96], F32, tag=f"s3{nm}", name=f"s3{nm}{b}")
                r3 = mp.tile([96, 96], F32, tag=f"r3{nm}", name=f"r3{nm}{b}")
                a_t = mp.tile([96, 288], BF16, tag=f"A{nm}", name=f"A{nm}{b}")
                bias = mp.tile([96, 96], BF16, tag=f"b{nm}", name=f"b{nm}{b}")
                for k in range(3):
                    for j in range(3):
                        pr = wp.tile([96, 1024], BF16, tag="prod")
                        qo, qd = qoff(k)
                        ko, kd = koff(j)
                        nc.vector.tensor_tensor(
                            pr[:, 0:1024],
                            _apv(src, qo, qd),
                            _apv(src, ko, kd),
                            MUL,
                        )
                        pair = 3 * k + j
                        nc.vector.tensor_reduce(
                            sraw[:, pair * 32:(pair + 1) * 32],
                            _apv(pr, 0, [[32, 32], [1, 32]]),
                            AXX, ADD,
                        )
                        yield
                nc.gpsimd.tensor_tensor(
                    _apv(sraw, 0, [[96, 3], [32, 3], [1, 32]]),
                    _apv(sraw, 0, [[96, 3], [32, 3], [1, 32]]),
                    _apv(src, sig[0], sig[1]),
                    ADD,
                )
                nc.scalar.activation(te[:], sraw[:], EXP)
                nc.vector.tensor_reduce(
                    _apv(s3, 0, [[32, 3], [1, 32]]),
                    _apv(te, 0, [[96, 3], [1, 32], [32, 3]]),
                    AXX, ADD,
                )
                yield
                nc.vector.reciprocal(r3[:], s3[:])
                nc.gpsimd.tensor_tensor(
                    _apv(a_t, 0, [[9, 32], [3, 3], [1, 3]]),
                    _apv(te, 0, [[1, 32], [96, 3], [32, 3]]),
                    _apv(r3, 0, [[1, 32], [32, 3], [0, 3]]),
                    MUL,
                )
                with nc.allow_low_precision("bias: sum of 3 bf16 weights"):
                    nc.vector.tensor_reduce(
                        _apv(bias, 0, [[3, 32], [1, 3]]),
                        _apv(a_t, 0, [[9, 32], [1, 3], [3, 3]]),
                        AXX, ADD,
                    )
                st[b]["A" + nm] = a_t
                st[b]["b" + nm] = bias
                yield

            def stage_scores_finish(b):
                """avtn transposes, btot = b_h^T + b_v (fp8), mv expansion."""
                a_v, b_h, b_v = st[b]["Av"], st[b]["bh"], st[b]["bv"]
                avtn = mp.tile([96, 288], BF16, tag="avtn", name=f"avtn{b}")
                btot = mp.tile([96, 96], FP8, tag="btot", name=f"btot{b}")
                mv = mp.tile([96, 9216], BF16, tag="mv", name=f"mv{b}")
                st[b]["avtn"], st[b]["btot"], st[b]["mv"] = avtn, btot, mv
                for j in range(3):
                    pt = pj.tile([96, 1024], BF16, tag="pj", name=f"ptn{b}_{j}")
                    nc.tensor.transpose(
                        pt[:, 0:96],
                        _apv(a_v, j, [[9, 32], [3, 3]]),
                        idtb_sb[:],
                    )
                    nc.vector.tensor_copy(
                        _apv(avtn, j, [[3, 96]]),
                        pt[:, 0:96],
                    )
                ptb = pj.tile([96, 1024], BF16, tag="pj", name=f"ptb{b}")
                nc.tensor.transpose(ptb[:, 0:96], b_h[:], idtb_sb[:])
                nc.vector.tensor_tensor(btot[:], ptb[:, 0:96], b_v[:], ADD)
                # mv[w, line*96 + (3m+j)] = pstr[w, 3m+j] * avtn[w, line*3+j]
                for q4 in range(4):
                    nc.vector.tensor_tensor(
                        _apv(mv, q4 * 24 * 96, [[96, 24], [3, 32], [1, 3]]),
                        _apv(pstr_sb, 0, [[0, 24], [3, 32], [1, 3]]),
                        _apv(avtn, q4 * 24 * 3, [[3, 24], [0, 32], [1, 3]]),
                        MUL,
                    )

            def stage_av_group(b, grp):
                """2 bands per rhs tile; per band: diag expansion (GPS
                affine_select or DVE ipat-TT) + mv add (DVE); per (band,cc):
                one fp8 DoubleRow matmul (k=0,1) + one plain fp8 matmul (k=2)
                into [128,1024] psum (bands at col 0/512); evict = ACT copy
                into bf16 staging, DMA out per 2 groups."""
                a_h, mv, btot = st[b]["Ah"], st[b]["mv"], st[b]["btot"]
                v_sb = st[b]["v"]
                n0 = 2 * grp
                rhs = rhs_tiles[grp % 2]
                for nb in range(2):
                    n = n0 + nb
                    nc.sync.dma_start(
                        _apv(rhs[96:97, :], nb * 864, [[1, 288]]),
                        btot[3 * n:3 * n + 3, :],
                    )
                    if b == 1 and nb == 1:
                        nc.vector.tensor_tensor(
                            _apv(rhs[0:96, :], nb * 864, [[96, 9], [1, 96]]),
                            _apv(ipat_sb, 0, [[96, 9], [1, 96]]),
                            _apv(a_h, n * 9, [[1, 9], [0, 96]]),
                            MUL,
                        )
                    else:
                        nc.gpsimd.affine_select(
                            _apv(rhs[0:96, :], nb * 864, [[1, 864]]),
                            _apv(a_h, n * 9, [[1, 9], [0, 96]]),
                            pattern=[[0, 9], [1, 96]],
                            compare_op=mybir.AluOpType.is_equal,
                            fill=0.0,
                            base=0,
                            channel_multiplier=-1,
                        )
                    nc.vector.tensor_tensor(
                        _apv(rhs[0:96, :], nb * 864, [[384, 3], [1, 96]]),
                        _apv(rhs[0:96, :], nb * 864, [[384, 3], [1, 96]]),
                        _apv(mv, 3 * n * 96, [[96, 3], [1, 96]]),
                        ADD,
                    )
                pso = {}
                for cc in range(2):
                    pso[cc] = avp.tile([128, 1024], F32, tag="av", bufs=2,
                                       name=f"av{cc}_{b}_{grp}")
                for nb in range(2):
                    n = n0 + nb
                    for cc in range(2):
                        ps = pso[cc]
                        po = 512 * nb
                        vt = v_sb[:].tensor
                        vo = v_sb[:].offset
                        nc.tensor.matmul(
                            _apv(ps, po, [[1, 288]]),
                            AP(vt, vo + (3 * n) * 256 + cc * 128,
                               [[96 * 256, 128], [256, 2], [1, 128]]),
                            _apv(rhs, nb * 864, [[288, 2], [1, 288]]),
                            start=True, stop=False,
                            perf_mode=DR,
                        )
                        nc.tensor.matmul(
                            _apv(ps, po, [[1, 288]]),
                            AP(vt, vo + (3 * n + 2) * 256 + cc * 128,
                               [[96 * 256, 128], [1, 128]]),
                            _apv(rhs, nb * 864 + 576, [[1, 288]]),
                            start=False, stop=True,
                        )
                if grp % 2 == 0:
                    for cc in range(2):
                        st[b][f"stage{cc}"] = wp.tile(
                            [128, 1152], BF16, tag=f"stage{cc}", bufs=2,
                            name=f"stage{cc}_{b}_{grp}")
                for cc in range(2):
                    nc.scalar.activation(
                        st[b][f"stage{cc}"][:, (grp % 2) * 576:(grp % 2) * 576 + 576],
                        _apv(pso[cc], 0, [[512, 2], [1, 288]]),
                        COPY,
                    )

            def stage_out_dma(b, qgrp):
                for cc in range(2):
                    nc.sync.dma_start(
                        out_e[b, cc, :, qgrp * 1152:(qgrp + 1) * 1152],
                        st[b][f"stage{cc}"][:],
                    )

            # ---------------- emission ----------------
            stage_load(0)
            stage_proj(0)
            stage_qkT(0)
            stage_load(1)
            g0 = itertools.chain(scores_products_gen(0, "h"),
                                 scores_products_gen(0, "v"))
            stage_proj(1, gen=g0)
            stage_qkT(1)
            stage_scores_finish(0)
            sg1 = itertools.chain(scores_products_gen(1, "h"),
                                  scores_products_gen(1, "v"))
            for grp in range(16):
                stage_av_group(0, grp)
                if grp % 2 == 1:
                    stage_out_dma(0, grp // 2)
                if grp >= 5:
                    next(sg1, None)
                    next(sg1, None)
            for _ in sg1:
                pass
            stage_scores_finish(1)
            for grp in range(16):
                stage_av_group(1, grp)
                if grp % 2 == 1:
                    stage_out_dma(1, grp // 2)
    nc.compile()
    return nc


def _host_prep(x, Wq, bq, Wk, bk, Wv, bv, gamma):
    x = np.ascontiguousarray(x, np.float32)
    sig_w = (bq @ Wk).astype(np.float32)          # [256]
    pad = np.zeros((QKC - 65, 256), np.float32)
    wall = np.concatenate([Wq, Wk, sig_w[None], pad, Wv], 0) * WSCALE  # [322, 256]
    # wall_e [128, 2, PW]: wall_e[c, cc, col] = wall[col, cc*128 + c]
    wallT = np.ascontiguousarray(
        wall.T.reshape(2, 128, PW).transpose(1, 0, 2)
    ).astype(ml_dtypes.float8_e4m3fn)
    pstr = np.kron(np.eye(32), np.ones((3, 3))).astype(ml_dtypes.bfloat16)
    ipat = np.tile(np.eye(96), (1, 9)).astype(ml_dtypes.bfloat16)
    idtb = np.eye(96).astype(ml_dtypes.bfloat16)
    gv = float(np.asarray(gamma).reshape(-1)[0])
    bvz = np.zeros((32, 96 * 256), np.float32)
    bvz[0] = np.tile(bv.astype(np.float32) * (VSCALE * gv), 96)
    bvz = bvz.astype(ml_dtypes.float8_e4m3fn)
    # xa_e [BPC, 128, 2, S]: xa[b, c, cc, pos] = x[b, cc*128 + c, pos]
    xr = x.reshape(B, 2, 128, S).transpose(0, 2, 1, 3)
    xf = np.ascontiguousarray(xr).astype(ml_dtypes.float8_e4m3fn)
    in_maps = []
    for i in range(NCORE):
        in_maps.append({
            "xa": xf[i * BPC:(i + 1) * BPC],
            "wall": wallT,
            "pstr": pstr, "ipat": ipat, "idtb": idtb,
            "bvz": bvz,
        })
    return in_maps


_CACHE = {}


def kernel(x, Wq, bq, Wk, bk, Wv, bv, gamma, _trace=False):
    x = np.asarray(x, np.float32)
    in_maps = _host_prep(x, np.asarray(Wq, np.float32), np.asarray(bq, np.float32),
                         np.asarray(Wk, np.float32), np.asarray(bk, np.float32),
                         np.asarray(Wv, np.float32), np.asarray(bv, np.float32),
                         np.asarray(gamma, np.float32))
    gv = float(np.asarray(gamma).reshape(-1)[0])
    if _CACHE.get("gamma") != gv:
        _CACHE["nc"] = build_graph(gv)
        _CACHE["gamma"] = gv
    nc = _CACHE["nc"]
    res = run_bass_kernel_spmd(nc, in_maps, list(range(NCORE)), trace=_trace)
    kernel.last_result = res
    out = np.empty((B, C, H, W), np.float32)
    for i in range(NCORE):
        o = np.asarray(res.results[i]["out"], np.float32)   # [BPC, 2, 128, S]
        for b in range(BPC):
            bi = i * BPC + b
            out[bi] = o[b].reshape(C, H, W) * (1.0 / VSCALE) + x[bi]
    return out


if __name__ == "__main__":
    rng = np.random.default_rng(0)
    xs = {k: rng.standard_normal(s).astype(np.float32) * (0.05 if k != "x" else 1.0)
          for k, s in [("x", (16, 256, 96, 96)), ("Wq", (32, 256)), ("bq", (32,)),
                       ("Wk", (32, 256)), ("bk", (32,)), ("Wv", (256, 256)),
                       ("bv", (256,)), ("gamma", (1,))]}
    y = kernel(**xs)
    print("ran", y.shape)


# revision 23
# speedup vs baseline: 1.1983x; 1.1983x over previous
import sys
import itertools

for p in ("/opt/trn_rl_repo",):
    if p not in sys.path:
        sys.path.insert(0, p)

import numpy as np
import ml_dtypes

from concourse import bass, mybir, bacc, tile
from concourse.ap import AP
from concourse.bass_utils import run_bass_kernel_spmd


def _install_ntff_hook():
    try:
        from antenv import axon_hooks  # noqa: F401
        return
    except ImportError:
        pass
    import types
    try:
        import antenv
    except ImportError:
        return
    mod = types.ModuleType("antenv.axon_hooks")
    _h = {"hook": None}
    mod.set_axon_ntff_profile_hook = lambda h: _h.__setitem__("hook", h)
    mod.get_axon_ntff_profile_hook = lambda: _h["hook"]
    sys.modules["antenv.axon_hooks"] = mod
    antenv.axon_hooks = mod
    try:
        from trn_agent_boot.trn_boot import _ntff_profile_via_ctypes
        h = _ntff_profile_via_ctypes("/opt/axon/libaxon_pjrt.so")
        if h is not None:
            mod.set_axon_ntff_profile_hook(h)
    except Exception:
        pass


_install_ntff_hook()


def _enable_ldw_opt():
    """walrus --enable-ldw-opt=false is hardcoded; flip it so LDWEIGHTS
    double-buffers against in-flight matmuls."""
    import concourse.bass_utils as _bu
    if getattr(_bu, "_ldw_patched", False):
        return
    _orig = _bu.run_command

    def _patched(argv, **kw):
        try:
            argv = ["--enable-ldw-opt=true" if c == "--enable-ldw-opt=false" else c
                    for c in argv]
        except TypeError:
            pass
        return _orig(argv, **kw)

    _bu.run_command = _patched
    _bu._ldw_patched = True


# NOTE: calling _enable_ldw_opt() crashes walrus (BackendPass C++ throw);
# leave the hardcoded --enable-ldw-opt=false in place.


F32 = mybir.dt.float32
BF16 = mybir.dt.bfloat16
FP8 = mybir.dt.float8e4
MUL = mybir.AluOpType.mult
ADD = mybir.AluOpType.add
AXX = mybir.AxisListType.X
EXP = mybir.ActivationFunctionType.Exp
COPY = mybir.ActivationFunctionType.Copy
DR = mybir.MatmulPerfMode.DoubleRow

B, C, H, W = 16, 256, 96, 96
S = H * W          # 9216
NCORE = 8
BPC = B // NCORE   # 2 batches per core
QKC = 66           # q(32) | k(32) | sigma(1) | pad(1)
PW = QKC + 256     # 322 proj width
HP = 128           # padded h for xbar transpose
WSCALE = 4.0       # wall rows prescale (fp8 range)
VSCALE = 16.0      # v_sb carries VSCALE*gamma*v


def _apv(t, off, dims):
    """Custom view on a tile/tensor AP: keep partition dim, custom free dims."""
    b = t[:] if not isinstance(t, AP) else t
    part = list(b.ap[0])
    return AP(b.tensor, b.offset + off, [part] + [list(d) for d in dims])


def build_graph(gamma):
    nc = bacc.Bacc(None, target_bir_lowering=False)
    vg = float(VSCALE * gamma / WSCALE)

    xa_e = nc.declare_dram_parameter("xa", [BPC, 128, 2, S], FP8, isOutput=False)
    wall_e = nc.declare_dram_parameter("wall", [128, 2, PW], FP8, isOutput=False)
    sidx_e = nc.declare_dram_parameter("sidx", [96, 36], mybir.dt.int16,
                                       isOutput=False)
    smsk_e = nc.declare_dram_parameter("smsk", [96, 4], BF16, isOutput=False)
    idtb_e = nc.declare_dram_parameter("idtb", [96, 96], BF16, isOutput=False)
    bvz_e = nc.declare_dram_parameter("bvz", [32, 96 * 256], FP8, isOutput=False)
    out_e = nc.declare_dram_parameter("out", [BPC, 2, 128, S], BF16, isOutput=True)

    with tile.TileContext(nc) as tc:
        with (
            tc.tile_pool(name="const", bufs=1) as cp,
            tc.tile_pool(name="main", bufs=1) as mp,
            tc.tile_pool(name="work", bufs=2) as wp,
            tc.tile_pool(name="pj", bufs=2, space="PSUM") as pj,
            tc.tile_pool(name="avp", bufs=2, space="PSUM") as avp,
        ):
            wall_sb = cp.tile([128, 2 * PW], FP8, tag="wall")
            nc.sync.dma_start(wall_sb[:], wall_e[:])
            sidx_sb = cp.tile([96, 36], mybir.dt.int16, tag="sidx")
            nc.sync.dma_start(sidx_sb[:], sidx_e[:])
            smsk_sb = cp.tile([96, 4], BF16, tag="smsk")
            nc.sync.dma_start(smsk_sb[:], smsk_e[:])
            idtb_sb = cp.tile([96, 96], BF16, tag="idtb")
            nc.sync.dma_start(idtb_sb[:], idtb_e[:])

            st = {0: {}, 1: {}}
            rhs_tiles = []
            for ri in range(2):
                rt = mp.tile([128, 2 * 864], BF16, tag=f"rhs{ri}")
                nc.vector.memset(rt[96:128, :], 0.0)
                rhs_tiles.append(rt)

            def stage_load(b):
                xa_sb = mp.tile([128, 2 * S], FP8, tag="xa", bufs=2,
                                name=f"xa{b}")
                nc.sync.dma_start(xa_sb[:], xa_e[b])
                st[b]["xa"] = xa_sb
                v_sb = mp.tile([128, 96 * 256], FP8, tag="v", bufs=2, name=f"v{b}")
                nc.sync.dma_start(v_sb[96:128, :], bvz_e[:])
                st[b]["v"] = v_sb
                qk_sb = mp.tile([96, QKC * HP], BF16, tag="qk", bufs=2,
                                name=f"qk{b}")
                st[b]["qk"] = qk_sb
                # pad rows h=96..127 of each channel (read by the xbar
                # transpose, discarded downstream)
                nc.gpsimd.memset(_apv(qk_sb, 96, [[HP, QKC], [1, HP - 96]]), 0.0)

            def stage_proj(b, gen=None):
                """proj: per 2 h-lines, one fp8 DoubleRow matmul each (cc pair
                fused as the two K-tiles); psum [96, 1024] (lines at col
                0/512); evict qk ch-major (ACT) + v (ACT/DVE split, fp8)."""
                xa_sb, qk_sb, v_sb = st[b]["xa"], st[b]["qk"], st[b]["v"]
                for g in range(48):
                    ps = pj.tile([96, 1024], F32, tag="pj", name=f"ps{b}_{g}")
                    for l2 in range(2):
                        h = 2 * g + l2
                        nc.tensor.matmul(
                            _apv(ps, 512 * l2, [[1, PW]]),
                            _apv(xa_sb, h * 96, [[S, 2], [1, 96]]),
                            _apv(wall_sb, 0, [[PW, 2], [1, PW]]),
                            start=True, stop=True,
                            perf_mode=DR,
                        )
                    # qk[w, c*HP + h] <- ps[w, l2*512 + c] / WSCALE
                    # (GPSIMD cannot read PSUM -> ACT only)
                    nc.scalar.activation(
                        _apv(qk_sb, 2 * g, [[HP, QKC], [1, 2]]),
                        _apv(ps, 0, [[1, QKC], [512, 2]]),
                        COPY, scale=1.0 / WSCALE,
                    )
                    if b == 0:
                        on_dve = g % 4 == 3
                    else:
                        on_dve = g % 8 == 7
                    if on_dve:
                        nc.vector.tensor_scalar_mul(
                            v_sb[0:96, g * 512:(g + 1) * 512],
                            _apv(ps, QKC, [[512, 2], [1, 256]]),
                            vg,
                        )
                    else:
                        nc.scalar.activation(
                            v_sb[0:96, g * 512:(g + 1) * 512],
                            _apv(ps, QKC, [[512, 2], [1, 256]]),
                            COPY, scale=vg,
                        )
                    if gen is not None and g % 2 == 1:
                        next(gen, None)
                if gen is not None:
                    for _ in gen:
                        pass

            def stage_qkT(b):
                """one XBAR dma-transpose: qk[w, c*HP+h] -> qkc[h, c*96+w]."""
                qk_sb = st[b]["qk"]
                qkc = mp.tile([128, QKC * 96], BF16, tag="qkc", bufs=2,
                              name=f"qkc{b}")
                st[b]["qkc"] = qkc
                nc.sync.dma_start_transpose(
                    _apv(qkc, 0, [[96, QKC], [1, 96]]),
                    qk_sb[:],
                )

            def scores_products_gen(b, nm):
                """nm='h': qk_sb [w, c*HP+h]; nm='v': qkc [h, c*96+w].
                products (DVE), reduces (DVE), sigma-add (GPS), exp (ACT),
                s3 (DVE), r3 (DVE), a (GPS), bias (DVE). Yields per pair."""
                if nm == "h":
                    src = st[b]["qk"][:]
                    cs = HP
                else:
                    src = st[b]["qkc"][0:96, :]
                    cs = 96
                qoff = lambda k: (k, [[3, 32], [cs, 32]])
                koff = lambda j: (32 * cs + j, [[3, 32], [cs, 32]])
                sig = (64 * cs, [[0, 3], [1, 3], [3, 32]])
                sraw = mp.tile([96, 288], F32, tag=f"sraw{nm}", name=f"sraw{nm}{b}")
                te = mp.tile([96, 288], F32, tag=f"te{nm}", name=f"te{nm}{b}")
                s3 = mp.tile([96, 96], F32, tag=f"s3{nm}", name=f"s3{nm}{b}")
                r3 = mp.tile([96, 96], F32, tag=f"r3{nm}", name=f"r3{nm}{b}")
                a_t = mp.tile([96, 288], BF16, tag=f"A{nm}", name=f"A{nm}{b}")
                bias = mp.tile([96, 96], BF16, tag=f"b{nm}", name=f"b{nm}{b}")
                for k in range(3):
                    # 3 j-pairs fused per op: pr[w, j*1024 + n*32 + c]
                    pr = wp.tile([96, 3072], BF16, tag="prod")
                    qo, qd = qoff(k)
                    ko, kd = koff(0)
                    eng = nc.gpsimd if (nm == "v" and k == 2) else nc.vector
                    eng.tensor_tensor(
                        pr[:, 0:3072],
                        _apv(src, qo, [[0, 3]] + qd),
                        _apv(src, ko, [[1, 3]] + kd),
                        MUL,
                    )
                    yield
                    nc.vector.tensor_reduce(
                        _apv(sraw, 3 * k * 32, [[32, 3], [1, 32]]),
                        _apv(pr, 0, [[1024, 3], [32, 32], [1, 32]]),
                        AXX, ADD,
                    )
                    yield
                nc.gpsimd.tensor_tensor(
                    _apv(sraw, 0, [[96, 3], [32, 3], [1, 32]]),
                    _apv(sraw, 0, [[96, 3], [32, 3], [1, 32]]),
                    _apv(src, sig[0], sig[1]),
                    ADD,
                )
                nc.scalar.activation(te[:], sraw[:], EXP)
                nc.vector.tensor_reduce(
                    _apv(s3, 0, [[32, 3], [1, 32]]),
                    _apv(te, 0, [[96, 3], [1, 32], [32, 3]]),
                    AXX, ADD,
                )
                yield
                nc.vector.reciprocal(r3[:], s3[:])
                nc.gpsimd.tensor_tensor(
                    _apv(a_t, 0, [[9, 32], [3, 3], [1, 3]]),
                    _apv(te, 0, [[1, 32], [96, 3], [32, 3]]),
                    _apv(r3, 0, [[1, 32], [32, 3], [0, 3]]),
                    MUL,
                )
                with nc.allow_low_precision("bias: sum of 3 bf16 weights"):
                    nc.vector.tensor_reduce(
                        _apv(bias, 0, [[3, 32], [1, 3]]),
                        _apv(a_t, 0, [[9, 32], [1, 3], [3, 3]]),
                        AXX, ADD,
                    )
                st[b]["A" + nm] = a_t
                st[b]["b" + nm] = bias
                yield

            def stage_scores_finish(b):
                """avtn transposes, btot = b_h^T + b_v, scatter-data build:
                sdat[w, 18n + (0..8)]  = a_h[w, 9n + i]           (diag values)
                sdat[w, 18n + (9..17)] = a_h[w, 9n+i]*msk + avtn  (band values)
                """
                a_v, b_h, b_v, a_h = st[b]["Av"], st[b]["bh"], st[b]["bv"], st[b]["Ah"]
                avtn = mp.tile([96, 288], BF16, tag="avtn", name=f"avtn{b}")
                btot = mp.tile([96, 96], BF16, tag="btot", name=f"btot{b}")
                sdat = mp.tile([96, 18 * 32], BF16, tag="sdat", name=f"sdat{b}")
                st[b]["avtn"], st[b]["btot"], st[b]["sdat"] = avtn, btot, sdat
                for j in range(3):
                    pt = pj.tile([96, 1024], BF16, tag="pj", name=f"ptn{b}_{j}")
                    nc.tensor.transpose(
                        pt[:, 0:96],
                        _apv(a_v, j, [[9, 32], [3, 3]]),
                        idtb_sb[:],
                    )
                    nc.vector.tensor_copy(
                        _apv(avtn, j, [[3, 96]]),
                        pt[:, 0:96],
                    )
                ptb = pj.tile([96, 1024], BF16, tag="pj", name=f"ptb{b}")
                nc.tensor.transpose(ptb[:, 0:96], b_h[:], idtb_sb[:])
                nc.vector.tensor_tensor(btot[:], ptb[:, 0:96], b_v[:], ADD)
                nc.vector.tensor_copy(
                    _apv(sdat, 0, [[18, 32], [1, 9]]),
                    _apv(a_h, 0, [[9, 32], [1, 9]]),
                )
                # band triple t=3j+jp: a_h[w,9n+4j]*msk[w,jp] + avtn[w,9n+t]
                nc.vector.tensor_tensor(
                    _apv(sdat, 9, [[18, 32], [3, 3], [1, 3]]),
                    _apv(a_h, 0, [[9, 32], [4, 3], [0, 3]]),
                    _apv(smsk_sb, 0, [[0, 32], [0, 3], [1, 3]]),
                    MUL,
                )
                nc.vector.tensor_tensor(
                    _apv(sdat, 9, [[18, 32], [1, 9]]),
                    _apv(sdat, 9, [[18, 32], [1, 9]]),
                    _apv(avtn, 0, [[9, 32], [1, 9]]),
                    ADD,
                )

            def stage_av_group(b, grp):
                """2 bands per rhs tile built by ONE gpsimd local_scatter
                (zeros + 36 sparse values/partition); per (band,cc): 3
                matmuls (fp8 v weights, bf16 rhs stream) into [128,1024]
                psum (bands at col 0/512); evict = ACT/DVE copy into bf16
                staging, DMA out per 2 groups."""
                sdat, btot = st[b]["sdat"], st[b]["btot"]
                v_sb = st[b]["v"]
                n0 = 2 * grp
                rhs = rhs_tiles[grp % 2]
                for nb in range(2):
                    n = n0 + nb
                    nc.sync.dma_start(
                        _apv(rhs[96:97, :], nb * 864, [[1, 288]]),
                        btot[3 * n:3 * n + 3, :],
                    )
                nc.gpsimd.local_scatter(
                    _apv(rhs[0:96, :], 0, [[1, 1728]]),
                    sdat[:, 36 * grp:36 * grp + 36],
                    sidx_sb[:],
                    channels=96, num_elems=1728, num_idxs=36,
                )
                pso = {}
                for cc in range(2):
                    pso[cc] = avp.tile([128, 1024], F32, tag="av", bufs=2,
                                       name=f"av{cc}_{b}_{grp}")
                for nb in range(2):
                    n = n0 + nb
                    for cc in range(2):
                        ps = pso[cc]
                        po = 512 * nb
                        vt = v_sb[:].tensor
                        vo = v_sb[:].offset
                        for k in range(3):
                            nc.tensor.matmul(
                                _apv(ps, po, [[1, 288]]),
                                AP(vt, vo + (3 * n + k) * 256 + cc * 128,
                                   [[96 * 256, 128], [1, 128]]),
                                _apv(rhs, nb * 864 + k * 288, [[1, 288]]),
                                start=(k == 0), stop=(k == 2),
                            )
                if grp % 2 == 0:
                    for cc in range(2):
                        st[b][f"stage{cc}"] = wp.tile(
                            [128, 1152], BF16, tag=f"stage{cc}", bufs=2,
                            name=f"stage{cc}_{b}_{grp}")
                for cc in range(2):
                    dst = st[b][f"stage{cc}"][:, (grp % 2) * 576:(grp % 2) * 576 + 576]
                    src = _apv(pso[cc], 0, [[512, 2], [1, 288]])
                    if grp % 4 == 3:
                        nc.vector.tensor_copy(dst, src)
                    else:
                        nc.scalar.activation(dst, src, COPY)

            def stage_out_dma(b, qgrp):
                for cc in range(2):
                    nc.sync.dma_start(
                        out_e[b, cc, :, qgrp * 1152:(qgrp + 1) * 1152],
                        st[b][f"stage{cc}"][:],
                    )

            # ---------------- emission ----------------
            stage_load(0)
            stage_proj(0)
            stage_qkT(0)
            stage_load(1)
            g0 = itertools.chain(scores_products_gen(0, "h"),
                                 scores_products_gen(0, "v"))
            stage_proj(1, gen=g0)
            stage_qkT(1)
            stage_scores_finish(0)
            sg1 = itertools.chain(scores_products_gen(1, "h"),
                                  scores_products_gen(1, "v"))
            for grp in range(16):
                stage_av_group(0, grp)
                if grp % 2 == 1:
                    stage_out_dma(0, grp // 2)
                if grp >= 5:
                    next(sg1, None)
                    next(sg1, None)
            for _ in sg1:
                pass
            stage_scores_finish(1)
            for grp in range(16):
                stage_av_group(1, grp)
                if grp % 2 == 1:
                    stage_out_dma(1, grp // 2)
    nc.compile()
    return nc


def _host_prep(x, Wq, bq, Wk, bk, Wv, bv, gamma):
    x = np.ascontiguousarray(x, np.float32)
    sig_w = (bq @ Wk).astype(np.float32)          # [256]
    pad = np.zeros((QKC - 65, 256), np.float32)
    wall = np.concatenate([Wq, Wk, sig_w[None], pad, Wv], 0) * WSCALE  # [322, 256]
    # wall_e [128, 2, PW]: wall_e[c, cc, col] = wall[col, cc*128 + c]
    wallT = np.ascontiguousarray(
        wall.T.reshape(2, 128, PW).transpose(1, 0, 2)
    ).astype(ml_dtypes.float8_e4m3fn)
    idtb = np.eye(96).astype(ml_dtypes.bfloat16)
    # scatter index table [96, 36] and band mask [96, 4]
    sidx = np.full((96, 36), -1, np.int16)
    for w in range(96):
        for nb in range(2):
            base, c0 = 864 * nb, 18 * nb
            for i in range(9):
                k, j = divmod(i, 3)
                if k != j:
                    sidx[w, c0 + i] = base + 96 * i + w
            for j in range(3):
                for jp in range(3):
                    sidx[w, c0 + 9 + 3 * j + jp] = base + 384 * j + 3 * (w // 3) + jp
    smsk = np.zeros((96, 4), np.float32)
    for w in range(96):
        smsk[w, w % 3] = 1.0
    smsk = smsk.astype(ml_dtypes.bfloat16)
    gv = float(np.asarray(gamma).reshape(-1)[0])
    bvz = np.zeros((32, 96 * 256), np.float32)
    bvz[0] = np.tile(bv.astype(np.float32) * (VSCALE * gv), 96)
    bvz = bvz.astype(ml_dtypes.float8_e4m3fn)
    # xa_e [BPC, 128, 2, S]: xa[b, c, cc, pos] = x[b, cc*128 + c, pos]
    xr = x.reshape(B, 2, 128, S).transpose(0, 2, 1, 3)
    xf = np.ascontiguousarray(xr).astype(ml_dtypes.float8_e4m3fn)
    in_maps = []
    for i in range(NCORE):
        in_maps.append({
            "xa": xf[i * BPC:(i + 1) * BPC],
            "wall": wallT,
            "sidx": sidx, "smsk": smsk, "idtb": idtb,
            "bvz": bvz,
        })
    return in_maps


_CACHE = {}


def kernel(x, Wq, bq, Wk, bk, Wv, bv, gamma, _trace=False):
    x = np.asarray(x, np.float32)
    in_maps = _host_prep(x, np.asarray(Wq, np.float32), np.asarray(bq, np.float32),
                         np.asarray(Wk, np.float32), np.asarray(bk, np.float32),
                         np.asarray(Wv, np.float32), np.asarray(bv, np.float32),
                         np.asarray(gamma, np.float32))
    gv = float(np.asarray(gamma).reshape(-1)[0])
    if _CACHE.get("gamma") != gv:
        _CACHE["nc"] = build_graph(gv)
        _CACHE["gamma"] = gv
    nc = _CACHE["nc"]
    res = run_bass_kernel_spmd(nc, in_maps, list(range(NCORE)), trace=_trace)
    kernel.last_result = res
    out = np.empty((B, C, H, W), np.float32)
    for i in range(NCORE):
        o = np.asarray(res.results[i]["out"], np.float32)   # [BPC, 2, 128, S]
        for b in range(BPC):
            bi = i * BPC + b
            out[bi] = o[b].reshape(C, H, W) * (1.0 / VSCALE) + x[bi]
    return out


if __name__ == "__main__":
    rng = np.random.default_rng(0)
    xs = {k: rng.standard_normal(s).astype(np.float32) * (0.05 if k != "x" else 1.0)
          for k, s in [("x", (16, 256, 96, 96)), ("Wq", (32, 256)), ("bq", (32,)),
                       ("Wk", (32, 256)), ("bk", (32,)), ("Wv", (256, 256)),
                       ("bv", (256,)), ("gamma", (1,))]}
    y = kernel(**xs)
    print("ran", y.shape)
